# revision 1
# baseline (speedup 1.0000x reference)
"""Trainium2 Bass kernel for nn_MultiHeadAttention (B=2, S=2048, D=1024, H=16).

Sharding (8 cores): batch (2-way) x head-group (4-way).
Core c: batch b=c//4, head-group hg=c%4 (4 heads = 256 of d_model).
Megatron style: Wq/Wk/Wv column-parallel, Wo row-parallel; the 4 partial
outputs per batch are summed on the host (plus b_o).

Per-core device pipeline (all matmuls f32r = TF32-like, 1 cyc/row):
  phase 1: project qhT/khT [do,t] and vh [t,do] from host-pre-transposed
           qT/kT/vT chunks; b_q/b_k fused as per-partition DVE adds, b_v
           commuted to a host-side b_v @ w_o output correction (exact).
  phase 2: per 512-query chunk x head-pair: scoresT[kj,qi] via K=64
           matmuls packed 2-heads-per-PE-pass (tile_position row strips),
           exp on ACT (scale 1/8 folded, one FD=2048 op per j-pair,
           fp16 out), keep-mask multiply on DVE (fp16 2x mode), PV
           accumulation with an appended ones
           column so row-sums ride along; normalization stays in [do, t]
           orientation: reciprocal of the sums row, K=1 outer-product
           broadcast across partitions on PE, one TT multiply.
  phase 3: o-proj into natural [t, d_model] layout, DMA out.
"""
import os

if "JAX_PLATFORMS" in os.environ and "axon" not in os.environ["JAX_PLATFORMS"]:
    del os.environ["JAX_PLATFORMS"]

import numpy as np
import ml_dtypes

B, S, D = 2, 2048, 1024
H, DK = 16, 64
NCORES = 8
HGROUPS = 4               # head-groups (cores per batch)
DLOC = D // HGROUPS       # 256 dims per core
NHL = DLOC // DK          # 4 local heads
NKT = D // 128            # 8 k-tiles over d_model
TCH = 512                 # token chunk
NCH = S // TCH            # 4 chunks
NT = S // 128             # 16 token tiles
NKJ = S // 128            # 16 key tiles
SCALE = 1.0 / 8.0         # 1/sqrt(DK)

_CACHE = {}


def _build(reps=1, parts=15):
    """Trace + compile the per-core Bass kernel (cached).

    reps>1 wraps the whole body in a tc.For_i hardware loop (timing use).
    parts: bitmask 1=phase1, 2=attention, 4=finalize, 8=oproj (bisection).
    """
    key = ("nc", reps, parts)
    if key in _CACHE:
        return _CACHE[key]
    import concourse.bacc as bacc
    import concourse.bass as bass
    import concourse.mybir as mybir
    from concourse.tile import TileContext

    f32r = mybir.dt.float32r
    f32 = mybir.dt.float32
    f16 = mybir.dt.float16
    AF = mybir.ActivationFunctionType

    nc = bacc.Bacc("TRN2", target_bir_lowering=False)

    qT_d = nc.dram_tensor("qT", [D, S], f32r, kind="ExternalInput")
    kT_d = nc.dram_tensor("kT", [D, S], f32r, kind="ExternalInput")
    vT_d = nc.dram_tensor("vT", [D, S], f32r, kind="ExternalInput")
    mk_d = nc.dram_tensor("maskT", [S, S], f16, kind="ExternalInput")
    wq_d = nc.dram_tensor("wq", [D, DLOC], f32r, kind="ExternalInput")
    wk_d = nc.dram_tensor("wk", [D, DLOC], f32r, kind="ExternalInput")
    wv_d = nc.dram_tensor("wv", [D, DLOC], f32r, kind="ExternalInput")
    wo_d = nc.dram_tensor("wo", [DLOC, D], f32r, kind="ExternalInput")
    bq_d = nc.dram_tensor("bq", [128, 2], f32r, kind="ExternalInput")
    bk_d = nc.dram_tensor("bk", [128, 2], f32r, kind="ExternalInput")
    ones2_d = nc.dram_tensor("ones2", [128, NT, NHL, 2], f16,
                             kind="ExternalInput")
    onesc_d = nc.dram_tensor("onesc", [1, DK], f32r, kind="ExternalInput")
    out_d = nc.dram_tensor("out", [S, D], f32, kind="ExternalOutput")

    qT_r = qT_d.rearrange("(kt p) t -> p kt t", p=128)
    kT_r = kT_d.rearrange("(kt p) t -> p kt t", p=128)
    vT_r = vT_d.rearrange("(kt p) t -> p kt t", p=128)
    mk_r = mk_d.rearrange("(j p) q -> p j q", p=128)

    with TileContext(nc) as tc:
        with (
            tc.tile_pool(name="big", bufs=1) as big,
            tc.tile_pool(name="xin", bufs=2) as xin,
            tc.tile_pool(name="mp", bufs=2) as mp,
            tc.tile_pool(name="ep", bufs=3) as ep,
            tc.tile_pool(name="sp", bufs=3) as sp,
            tc.tile_pool(name="ps", bufs=1, space="PSUM") as ps,
        ):
          import contextlib
          loop_cm = tc.For_i(0, reps, 1) if reps > 1 else contextlib.nullcontext()
          with loop_cm:
            # ---- constants / weights ----
            wq_sb = big.tile([128, NKT, DLOC], f32r)
            wk_sb = big.tile([128, NKT, DLOC], f32r)
            wv_sb = big.tile([128, NKT, DLOC], f32r)
            wo_sb = big.tile([128, DLOC // 128, D], f32r)
            nc.sync.dma_start(out=wq_sb, in_=wq_d.rearrange("(kt p) o -> p kt o", p=128))
            nc.sync.dma_start(out=wk_sb, in_=wk_d.rearrange("(kt p) o -> p kt o", p=128))
            nc.sync.dma_start(out=wv_sb, in_=wv_d.rearrange("(kt p) o -> p kt o", p=128))
            nc.sync.dma_start(out=wo_sb, in_=wo_d.rearrange("(kk p) o -> p kk o", p=128))
            bq_sb = big.tile([128, 2], f32r)
            bk_sb = big.tile([128, 2], f32r)
            onesc_sb = big.tile([1, DK], f32r)
            nc.sync.dma_start(out=bq_sb, in_=bq_d[:, :])
            nc.sync.dma_start(out=bk_sb, in_=bk_d[:, :])
            nc.sync.dma_start(out=onesc_sb, in_=onesc_d[:, :])

            # ---- persistent activations ----
            qhT_sb = big.tile([128, 2, S], f32r)     # [p, m, t]
            khT_sb = big.tile([128, 2, S], f32r)
            vh1_sb = big.tile([128, NT, NHL, DK + 2], f16)
            aoT_sb = big.tile([128, 2, S], f32r)     # normalized attnout^T
            nc.sync.dma_start(out=vh1_sb[:, :, :, DK:DK + 2], in_=ones2_d[:, :, :, :])

            # one 4-bank psum slot shared by qk-proj (quadrants) and scores
            s4 = ps.tile([128, 2, 2, TCH], f32, tag="s4", name="s4", bufs=1)

            # ---- phase 1: projections ----
            quad = 0

            def emit_qk_chunk(xname, xr, w_sb, b_sb, hT_sb, tch):
                nonlocal quad
                xt = xin.tile([128, NKT, TCH], f32r, tag="xt",
                              name=f"xt_{xname}{tch}")
                nc.sync.dma_start(
                    out=xt, in_=xr[:, :, tch * TCH:(tch + 1) * TCH])
                for m in range(2):
                    acc = s4[:, quad % 2, quad // 2 % 2, :]
                    for kt in range(NKT):
                        nc.tensor.matmul(
                            acc, w_sb[:, kt, m * 128:(m + 1) * 128],
                            xt[:, kt, :],
                            start=(kt == 0), stop=(kt == NKT - 1))
                    nc.vector.tensor_scalar_add(
                        out=hT_sb[:, m, tch * TCH:(tch + 1) * TCH],
                        in0=acc, scalar1=b_sb[:, m:m + 1].bitcast(f32))
                    quad += 1

            def emit_v_chunk(tch):
                xt = xin.tile([128, NKT, TCH], f32r, tag="xt", name=f"xt_v{tch}")
                nc.sync.dma_start(
                    out=xt, in_=vT_r[:, :, tch * TCH:(tch + 1) * TCH])
                for mm in range(TCH // 128):
                    m16 = tch * (TCH // 128) + mm
                    pv = ps.tile([128, DLOC], f32, tag="pvx",
                                 name=f"psv_{m16}", bufs=2)
                    for kt in range(NKT):
                        nc.tensor.matmul(
                            pv, xt[:, kt, mm * 128:(mm + 1) * 128],
                            wv_sb[:, kt, :],
                            start=(kt == 0), stop=(kt == NKT - 1))
                    nc.vector.tensor_copy(
                        vh1_sb[:, m16, :, 0:DK],
                        pv.rearrange("p (h d) -> p h d", h=NHL))

            if parts & 1:
                for tch in range(NCH):
                    emit_qk_chunk("k", kT_r, wk_sb, bk_sb, khT_sb, tch)
                    emit_v_chunk(tch)
                for tch in range(NCH):
                    emit_qk_chunk("q", qT_r, wq_sb, bq_sb, qhT_sb, tch)

            # ---- phase 2: attention ----
            LOOK = 2
            for tcq in range(NCH if parts & 2 else 0):
                qsl = slice(tcq * TCH, (tcq + 1) * TCH)
                mk_sb = mp.tile([128, NKJ, TCH], f16, tag="mk",
                                name=f"mk_{tcq}")
                nc.sync.dma_start(out=mk_sb, in_=mk_r[:, :, tcq * TCH:(tcq + 1) * TCH])
                for hp in range(2):
                    pvT2 = ps.tile([DK + 2, 2, TCH], f32, tag="pvx",
                                   name=f"pvT_{tcq}_{hp}", bufs=2)
                    s_sl = ps.tile([128, 2, 2, TCH], f32, tag="s4",
                                   name=f"s_{tcq}_{hp}", bufs=1)
                    e_tiles = {}
                    NJG = NKJ // 2
                    for jg in range(NJG + 1):
                        if jg < NJG:
                            e_sb = ep.tile([128, 2, 2, TCH], f16, tag="e",
                                           name=f"e_{tcq}_{hp}_{jg}", bufs=3)
                            e_tiles[jg] = e_sb
                            for jj in range(2):
                                j = jg * 2 + jj
                                for hh in range(2):
                                    nc.tensor.matmul(
                                        s_sl[:, jj, hh, :],
                                        khT_sb[64 * hh:64 * (hh + 1), hp,
                                               j * 128:(j + 1) * 128],
                                        qhT_sb[64 * hh:64 * (hh + 1), hp, qsl],
                                        start=True, stop=True,
                                        tile_position=(64 * hh, 0))
                            nc.scalar.activation(
                                out=e_sb, in_=s_sl,
                                func=AF.Exp, scale=SCALE)
                            msl = mk_sb[:, jg * 2:jg * 2 + 2, :]
                            mbc = bass.AP(
                                tensor=msl.tensor, offset=msl.offset,
                                ap=[msl.ap[0], msl.ap[1], [0, 2],
                                    msl.ap[2]])
                            nc.vector.tensor_mul(e_sb, e_sb, mbc)
                        jp = jg - 1
                        if jp >= 0:
                            e_c = e_tiles.pop(jp)
                            for jj in range(2):
                                jc = jp * 2 + jj
                                for hh in range(2):
                                    nc.tensor.matmul(
                                        pvT2[:, hh, :],
                                        vh1_sb[:, jc, hp * 2 + hh, :],
                                        e_c[:, jj, hh, :],
                                        start=(jc == 0),
                                        stop=(jc == NKJ - 1))
                    # finalize pair: normalize in [do, t] orientation:
                    # recip of the sums row, PE-broadcast across partitions
                    # (K=1 outer product), one TT multiply into aoT_sb.
                    if parts & 4:
                        pvT_sb = sp.tile([DK + 2, 2, TCH], f32r, tag="pvs",
                                         name=f"pvs_{tcq}_{hp}")
                        nc.vector.tensor_copy(pvT_sb, pvT2)
                        rec = sp.tile([1, 2, TCH], f32r, tag="rec",
                                      name=f"rec_{tcq}_{hp}")
                        with nc.allow_low_precision(
                                reason="recip row feeds f32r broadcast mm"):
                            nc.vector.reciprocal(rec, pvT_sb[DK:DK + 1, :, :])
                        for hh in range(2):
                            bc = ps.tile([DK, TCH], f32, tag="pvx",
                                         name=f"bc_{tcq}_{hp}_{hh}", bufs=2)
                            nc.tensor.matmul(bc, onesc_sb, rec[0:1, hh, :],
                                             start=True, stop=True)
                            nc.vector.tensor_mul(
                                aoT_sb[64 * hh:64 * (hh + 1), hp, qsl],
                                pvT_sb[0:DK, hh, :], bc)

            # ---- phase 3: o-proj ----
            for m16 in range(NT if parts & 8 else 0):
                o_sb = sp.tile([128, D], f32, tag="o", name=f"o_{m16}")
                po = ps.tile([128, 2, 512], f32, tag="pvx",
                             name=f"po_{m16}", bufs=2)
                for n in range(2):
                    for kk in range(2):
                        nc.tensor.matmul(
                            po[:, n, :],
                            aoT_sb[:, kk, m16 * 128:(m16 + 1) * 128],
                            wo_sb[:, kk, n * 512:(n + 1) * 512],
                            start=(kk == 0), stop=(kk == 1))
                nc.vector.tensor_copy(o_sb.rearrange("p (n q) -> p n q", n=2), po)
                nc.sync.dma_start(
                    out=out_d[m16 * 128:(m16 + 1) * 128, :], in_=o_sb)

    nc.compile()
    _CACHE[key] = nc
    return nc


def _in_maps(q, k, v, mask, w_q, b_q, w_k, b_k, w_v, b_v, w_o, b_o):
    q = np.asarray(q, dtype=np.float32)
    k = np.asarray(k, dtype=np.float32)
    v = np.asarray(v, dtype=np.float32)
    mask = np.asarray(mask)
    w_q = np.asarray(w_q, dtype=np.float32)
    w_k = np.asarray(w_k, dtype=np.float32)
    w_v = np.asarray(w_v, dtype=np.float32)
    w_o = np.asarray(w_o, dtype=np.float32)
    b_q = np.asarray(b_q, dtype=np.float32)
    b_k = np.asarray(b_k, dtype=np.float32)
    b_v = np.asarray(b_v, dtype=np.float32)

    hf = np.float16
    qT = [np.ascontiguousarray(q[b].T) for b in range(B)]
    kT = [np.ascontiguousarray(k[b].T) for b in range(B)]
    vT = [np.ascontiguousarray(v[b].T) for b in range(B)]
    mkT = [np.ascontiguousarray((~mask[b, 0]).T).astype(hf) for b in range(B)]
    ones2 = np.ones((128, NT, NHL, 2), dtype=hf)

    maps = []
    for c in range(NCORES):
        b, hg = c // HGROUPS, c % HGROUPS
        sl = slice(hg * DLOC, (hg + 1) * DLOC)
        maps.append({
            "qT": qT[b], "kT": kT[b], "vT": vT[b], "maskT": mkT[b],
            "wq": np.ascontiguousarray(w_q[:, sl]),
            "wk": np.ascontiguousarray(w_k[:, sl]),
            "wv": np.ascontiguousarray(w_v[:, sl]),
            "wo": np.ascontiguousarray(w_o[sl, :]),
            "bq": np.ascontiguousarray(b_q[sl].reshape(2, 128).T),
            "bk": np.ascontiguousarray(b_k[sl].reshape(2, 128).T),
            "ones2": ones2,
            "onesc": np.ones((1, DK), dtype=np.float32),
        })
    return maps


def kernel(q, k, v, mask, w_q, b_q, w_k, b_k, w_v, b_v, w_o, b_o):
    from concourse.bass_utils import run_bass_kernel_spmd

    nc = _build()
    maps = _in_maps(q, k, v, mask, w_q, b_q, w_k, b_k, w_v, b_v, w_o, b_o)
    res = run_bass_kernel_spmd(nc, maps, list(range(NCORES)))
    b_o = np.asarray(b_o, dtype=np.float32)
    out = np.zeros((B, S, D), dtype=np.float32)
    for c in range(NCORES):
        out[c // HGROUPS] += res.results[c]["out"]
    out += b_o + (np.asarray(b_v, dtype=np.float32) @
                  np.asarray(w_o, dtype=np.float32))
    return out



# revision 14
# speedup vs baseline: 8.8159x; 8.8159x over previous
"""Trainium2 Bass kernel for nn_MultiHeadAttention (B=2, S=2048, D=1024, H=16).

Sharding (8 cores): batch (2-way) x head-group (4-way).
Core c: batch b=c//4, head-group hg=c%4 (4 heads = 256 of d_model).
Megatron style: Wq/Wk/Wv column-parallel, Wo row-parallel; the 4 partial
outputs per batch are summed on the host (plus b_o).

Per-core device pipeline (all matmuls f32r = TF32-like, 1 cyc/row):
  DMA order tuned for earliest attention start: wk,wq -> k chunks -> q0
  -> mask0 -> wv -> v chunks; later q chunks / masks / wo prefetched
  during attention.

  Attention is a single software-pipelined stream per (query-chunk,
  head-pair): per 128-key tile j: scoresT[kj,qi] via two K=64 matmuls
  packed 2-heads-per-PE-pass (tile_position row strips) into a 2-bank
  PSUM tile (double-buffered), exp on ACT (scale 1/8 folded, fp16 out),
  keep-mask multiply on DVE (fp16 2x, head-broadcast), PV accumulation
  (ones column rides along for row sums) lagging one j behind.

  The ACT engine is the steady-state bottleneck (~1.04us per j), so all
  other work is interleaved into the j-loop as PE filler at fixed j
  positions: normalization of the previous head-pair (reciprocal of the
  PSUM sums row, K=1 outer-product broadcast on PE, TT multiply),
  o-projection of the previous query chunk, and q-projection of the
  next. This keeps PE dense (p-state at full clock) and hides
  everything under the exp stream.
"""
import os

if "JAX_PLATFORMS" in os.environ and "axon" not in os.environ["JAX_PLATFORMS"]:
    del os.environ["JAX_PLATFORMS"]

import numpy as np
import ml_dtypes

B, S, D = 2, 2048, 1024
H, DK = 16, 64
NCORES = 8
HGROUPS = 4               # head-groups (cores per batch)
DLOC = D // HGROUPS       # 256 dims per core
NHL = DLOC // DK          # 4 local heads
NKT = D // 128            # 8 k-tiles over d_model
TCH = 512                 # token chunk
NCH = S // TCH            # 4 chunks
NT = S // 128             # 16 token tiles
NKJ = S // 128            # 16 key tiles
SCALE = 1.0 / 8.0         # 1/sqrt(DK)

_CACHE = {}


def _build(reps=1, parts=15):
    """Trace + compile the per-core Bass kernel (cached).

    reps>1 wraps the whole body in a tc.For_i hardware loop (timing use).
    parts: bitmask 1=phase1, 2=attention, 4=finalize, 8=oproj (bisection).
    """
    key = ("nc", reps, parts)
    if key in _CACHE:
        return _CACHE[key]
    import concourse.bacc as bacc
    import concourse.bass as bass
    import concourse.mybir as mybir
    from concourse.tile import TileContext

    f32r = mybir.dt.float32r
    f32 = mybir.dt.float32
    f16 = mybir.dt.float16
    AF = mybir.ActivationFunctionType

    nc = bacc.Bacc("TRN2", target_bir_lowering=False)

    qT_d = nc.dram_tensor("qT", [D, S], f32r, kind="ExternalInput")
    kT_d = nc.dram_tensor("kT", [D, S], f32r, kind="ExternalInput")
    vT_d = nc.dram_tensor("vT", [D, S], f32r, kind="ExternalInput")
    mk_d = nc.dram_tensor("maskT", [S, S], f16, kind="ExternalInput")
    wq_d = nc.dram_tensor("wq", [D, DLOC], f32r, kind="ExternalInput")
    wk_d = nc.dram_tensor("wk", [D, DLOC], f32r, kind="ExternalInput")
    wv_d = nc.dram_tensor("wv", [D, DLOC], f32r, kind="ExternalInput")
    wo_d = nc.dram_tensor("wo", [DLOC, D], f32r, kind="ExternalInput")
    bq_d = nc.dram_tensor("bq", [128, 2], f32r, kind="ExternalInput")
    bk_d = nc.dram_tensor("bk", [128, 2], f32r, kind="ExternalInput")
    ones2_d = nc.dram_tensor("ones2", [128, NT, NHL, 2], f16,
                             kind="ExternalInput")
    onesc_d = nc.dram_tensor("onesc", [1, DK], f32r, kind="ExternalInput")
    out_d = nc.dram_tensor("out", [S, D], f32, kind="ExternalOutput")

    qT_r = qT_d.rearrange("(kt p) t -> p kt t", p=128)
    kT_r = kT_d.rearrange("(kt p) t -> p kt t", p=128)
    vT_r = vT_d.rearrange("(kt p) t -> p kt t", p=128)
    mk_r = mk_d.rearrange("(j p) q -> p j q", p=128)

    with TileContext(nc) as tc:
        with (
            tc.tile_pool(name="big", bufs=1) as big,
            tc.tile_pool(name="xin", bufs=2) as xin,
            tc.tile_pool(name="mp", bufs=2) as mp,
            tc.tile_pool(name="ep", bufs=6) as ep,
            tc.tile_pool(name="sp", bufs=2) as sp,
            tc.tile_pool(name="ps", bufs=1, space="PSUM") as ps,
        ):
          import contextlib
          loop_cm = tc.For_i(0, reps, 1) if reps > 1 else contextlib.nullcontext()
          with loop_cm:
            # ---- weights / constants (DMA order = priority order) ----
            wq_sb = big.tile([128, NKT, DLOC], f32r)
            wk_sb = big.tile([128, NKT, DLOC], f32r)
            wv_sb = big.tile([128, NKT, DLOC], f32r)
            wo_sb = big.tile([128, DLOC // 128, D], f32r)
            bq_sb = big.tile([128, 2], f32r)
            bk_sb = big.tile([128, 2], f32r)
            onesc_sb = big.tile([1, DK], f32r)
            nc.sync.dma_start(out=wk_sb, in_=wk_d.rearrange("(kt p) o -> p kt o", p=128))
            nc.sync.dma_start(out=wq_sb, in_=wq_d.rearrange("(kt p) o -> p kt o", p=128))
            nc.sync.dma_start(out=bk_sb, in_=bk_d[:, :])
            nc.sync.dma_start(out=bq_sb, in_=bq_d[:, :])

            # ---- persistent activations ----
            qhT_sb = big.tile([128, 2, S], f32r)     # [p, m, t]
            khT_sb = big.tile([128, 2, S], f32r)
            vh1_sb = big.tile([128, NT, NHL, DK + 2], f16)
            aoT_sb = big.tile([128, 2, S], f32r)     # normalized attnout^T

            # PSUM budget (8 banks):
            #   sj  tag: [128, 2, TCH] f32 = 2 banks x 2 bufs  (scores,
            #       qk-proj accumulators, normalize broadcast)
            #   pv2 tag: [66, 2, TCH]  f32 = 2 banks x 1 buf   (PV accum)
            #   po  tag: [128, 2, 512] f32 = 2 banks x 1 buf   (v-proj,
            #       o-proj, interleaved q-proj accumulators)

            def emit_x_dma(xname, xr, tch):
                xt = xin.tile([128, NKT, TCH], f32r, tag="xt",
                              name=f"xt_{xname}{tch}")
                nc.sync.dma_start(
                    out=xt, in_=xr[:, :, tch * TCH:(tch + 1) * TCH])
                return xt

            def emit_qk_compute(xt, w_sb, b_sb, hT_sb, tch, ms, label,
                                acc_tag="sj", acc_bufs=2):
                acc = ps.tile([128, 2, TCH], f32, tag=acc_tag,
                              name=f"acc_{label}_{tch}_{ms[0]}",
                              bufs=acc_bufs)
                for i, m in enumerate(ms):
                    for kt in range(NKT):
                        nc.tensor.matmul(
                            acc[:, i, :], w_sb[:, kt, m * 128:(m + 1) * 128],
                            xt[:, kt, :],
                            start=(kt == 0), stop=(kt == NKT - 1))
                for i, m in enumerate(ms):
                    nc.vector.tensor_scalar_add(
                        out=hT_sb[:, m, tch * TCH:(tch + 1) * TCH],
                        in0=acc[:, i, :],
                        scalar1=b_sb[:, m:m + 1].bitcast(f32))

            def emit_v_compute(xt, tch):
                for mm in range(TCH // 128):
                    m16 = tch * (TCH // 128) + mm
                    pv = ps.tile([128, 2, 512], f32, tag="po",
                                 name=f"psv_{m16}", bufs=1)
                    for kt in range(NKT):
                        nc.tensor.matmul(
                            pv[:, 0, 0:DLOC],
                            xt[:, kt, mm * 128:(mm + 1) * 128],
                            wv_sb[:, kt, :],
                            start=(kt == 0), stop=(kt == NKT - 1))
                    nc.vector.tensor_copy(
                        vh1_sb[:, m16, :, 0:DK],
                        pv[:, 0, 0:DLOC].rearrange("p (h d) -> p h d", h=NHL))

            # ---- phase 1: chunk-interleaved so attention can start as
            # soon as (k0, q0, mask0, v0) are resident; all projection
            # accumulators share the serial DMA-paced "po" psum slot so
            # the "sj" slots stay free for the attention scores rhythm.
            mk0_sb = mp.tile([128, NKJ, TCH], f16, tag="mk", name="mk_0")
            if parts & 1:
                kx0 = emit_x_dma("k", kT_r, 0)
                qx0 = emit_x_dma("q", qT_r, 0)
                emit_qk_compute(kx0, wk_sb, bk_sb, khT_sb, 0, [0, 1], "k",
                                acc_tag="po", acc_bufs=1)
                nc.sync.dma_start(out=mk0_sb, in_=mk_r[:, :, 0:TCH])
                nc.sync.dma_start(out=wv_sb,
                                  in_=wv_d.rearrange("(kt p) o -> p kt o", p=128))
                vx0 = emit_x_dma("v", vT_r, 0)
                nc.sync.dma_start(out=vh1_sb[:, :, :, DK:DK + 2],
                                  in_=ones2_d[:, :, :, :])
                nc.sync.dma_start(out=onesc_sb, in_=onesc_d[:, :])
                emit_qk_compute(qx0, wq_sb, bq_sb, qhT_sb, 0, [0, 1], "q",
                                acc_tag="po", acc_bufs=1)
                emit_v_compute(vx0, 0)
                for tch in range(1, NCH):
                    kxt = emit_x_dma("k", kT_r, tch)
                    vxt = emit_x_dma("v", vT_r, tch)
                    emit_qk_compute(kxt, wk_sb, bk_sb, khT_sb, tch, [0, 1],
                                    "k", acc_tag="po", acc_bufs=1)
                    emit_v_compute(vxt, tch)
            else:
                nc.sync.dma_start(out=mk0_sb, in_=mk_r[:, :, 0:TCH])
                nc.sync.dma_start(out=wv_sb,
                                  in_=wv_d.rearrange("(kt p) o -> p kt o", p=128))
                nc.sync.dma_start(out=vh1_sb[:, :, :, DK:DK + 2],
                                  in_=ones2_d[:, :, :, :])
                nc.sync.dma_start(out=onesc_sb, in_=onesc_d[:, :])

            nc.sync.dma_start(out=wo_sb, in_=wo_d.rearrange("(kk p) o -> p kk o", p=128))

            # ---- attention: one flat pipeline over (stream, j) columns
            # so the exp stream never drains at head-pair boundaries.
            # streams s = (tcq, hp); per column c = 16*s + j:
            #   scores/exp/mask(c), then lag-2 PV(c-2), then PE fillers
            #   (normalize of s-1, o-proj of tcq-1, q-proj of tcq+1).
            streams = [(tcq, hp) for tcq in range(NCH) for hp in range(2)]
            NS = len(streams)
            mk_tiles = {0: mk0_sb}
            state = {"qxt": None}
            pvT2s = {}
            e_tiles = {}

            def emit_finalize(s):
                # normalize straight out of PSUM: recip of the sums row
                # (emitted right after PV(s,15)), K=1 outer-product
                # broadcast on PE, one TT multiply per head into aoT_sb.
                tcq, hp = streams[s]
                qsl = slice(tcq * TCH, (tcq + 1) * TCH)
                rec = state[("rec", s)]
                pvT2 = pvT2s.pop(s)
                bc = ps.tile([DK, 2, TCH], f32, tag="po",
                             name=f"bc_{tcq}_{hp}", bufs=1)
                for hh in range(2):
                    nc.tensor.matmul(bc[:, hh, :], onesc_sb,
                                     rec[0:1, hh, :],
                                     start=True, stop=True)
                # DVE TensorTensor may read at most one PSUM input, so
                # stage the broadcast in SBUF before the multiply
                bc_sb = sp.tile([DK, 2, TCH], f32r, tag="bcs",
                                name=f"bcs_{tcq}_{hp}", bufs=2)
                nc.vector.tensor_copy(bc_sb, bc)
                for hh in range(2):
                    nc.vector.tensor_mul(
                        aoT_sb[64 * hh:64 * (hh + 1), hp, qsl],
                        pvT2[0:DK, hh, :], bc_sb[:, hh, :])

            def emit_oproj_half(m16, half, tag="po", bufs=1):
                # split across two filler columns to stay inside the
                # per-column PE slack
                if half == 0:
                    po = ps.tile([128, 2, 512], f32, tag=tag,
                                 name=f"po_{m16}", bufs=bufs)
                    state[("po", m16)] = po
                else:
                    po = state.pop(("po", m16))
                for kk in range(2):
                    nc.tensor.matmul(
                        po[:, half, :],
                        aoT_sb[:, kk, m16 * 128:(m16 + 1) * 128],
                        wo_sb[:, kk, half * 512:(half + 1) * 512],
                        start=(kk == 0), stop=(kk == 1))
                if half == 1:
                    o_sb = sp.tile([128, D], f32, tag="o", name=f"o_{m16}",
                                   bufs=2)
                    nc.vector.tensor_copy(
                        o_sb.rearrange("p (n q) -> p n q", n=2), po)
                    nc.sync.dma_start(
                        out=out_d[m16 * 128:(m16 + 1) * 128, :], in_=o_sb)

            def emit_col(s, j):
                tcq, hp = streams[s]
                qsl = slice(tcq * TCH, (tcq + 1) * TCH)
                s_j = ps.tile([128, 2, TCH], f32, tag="sj",
                              name=f"s_{tcq}_{hp}_{j}", bufs=2)
                for hh in range(2):
                    nc.tensor.matmul(
                        s_j[:, hh, :],
                        khT_sb[64 * hh:64 * (hh + 1), hp,
                               j * 128:(j + 1) * 128],
                        qhT_sb[64 * hh:64 * (hh + 1), hp, qsl],
                        start=True, stop=True,
                        tile_position=(64 * hh, 0))
                e_sb = ep.tile([128, 2, TCH], f16, tag="e",
                               name=f"e_{tcq}_{hp}_{j}", bufs=8)
                e_tiles[16 * s + j] = e_sb
                nc.scalar.activation(
                    out=e_sb, in_=s_j, func=AF.Exp, scale=SCALE)
                mk_sb = mk_tiles[tcq]
                msl = mk_sb[:, j, :]
                mbc = bass.AP(
                    tensor=msl.tensor, offset=msl.offset,
                    ap=[msl.ap[0], [0, 2], msl.ap[1]])
                nc.vector.tensor_mul(e_sb, e_sb, mbc)

            def emit_pv(s, j):
                tcq, hp = streams[s]
                if j == 0:
                    pvT2s[s] = ps.tile([DK + 2, 2, TCH], f32, tag="pv2",
                                       name=f"pvT_{tcq}_{hp}", bufs=1)
                pvT2 = pvT2s[s]
                e_c = e_tiles.pop(16 * s + j)
                for hh in range(2):
                    nc.tensor.matmul(
                        pvT2[:, hh, :],
                        vh1_sb[:, j, hp * 2 + hh, :],
                        e_c[:, hh, :],
                        start=(j == 0), stop=(j == NKJ - 1))
                if j == NKJ - 1 and parts & 4:
                    rec = sp.tile([1, 2, TCH], f32r, tag="rec",
                                  name=f"rec_{tcq}_{hp}")
                    with nc.allow_low_precision(
                            reason="recip row feeds f32r broadcast mm"):
                        nc.vector.reciprocal(rec, pvT2[DK:DK + 1, :, :])
                    state[("rec", s)] = rec

            def fillers(s, j):
                tcq, hp = streams[s]
                if hp == 0 and j == 14 and tcq + 1 < NCH:
                    mk = mp.tile([128, NKJ, TCH], f16, tag="mk",
                                 name=f"mk_{tcq + 1}")
                    nc.sync.dma_start(
                        out=mk,
                        in_=mk_r[:, :, (tcq + 1) * TCH:(tcq + 2) * TCH])
                    mk_tiles[tcq + 1] = mk
                if parts & 8 and j in (6, 7, 10, 11) and tcq > 0:
                    emit_oproj_half((tcq - 1) * 4 + hp * 2 + (j >= 10),
                                    half=int(j in (7, 11)))
                if parts & 1 and j == 9 and hp == 0 and tcq + 1 < NCH:
                    state["qxt"] = emit_x_dma("q", qT_r, tcq + 1)
                if parts & 1 and j >= 12 and tcq + 1 < NCH:
                    # 2 of the 8 K-tiles per column, same accumulation group
                    kts = range(2 * (j - 12), 2 * (j - 12) + 2)
                    emit_qk_part(state["qxt"], tcq + 1, hp, kts, j == 12)

            def emit_qk_part(xt, tch, m, kts, first):
                if first:
                    state[("qacc", m)] = ps.tile(
                        [128, 2, TCH], f32, tag="po",
                        name=f"qacc_{tch}_{m}", bufs=1)
                acc = state[("qacc", m)]
                for kt in kts:
                    nc.tensor.matmul(
                        acc[:, 0, :], wq_sb[:, kt, m * 128:(m + 1) * 128],
                        xt[:, kt, :],
                        start=(kt == 0), stop=(kt == NKT - 1))
                if kts[-1] == NKT - 1:
                    state.pop(("qacc", m))
                    nc.vector.tensor_scalar_add(
                        out=qhT_sb[:, m, tch * TCH:(tch + 1) * TCH],
                        in0=acc[:, 0, :],
                        scalar1=bq_sb[:, m:m + 1].bitcast(f32))

            if parts & 2:
                # flat pipeline with a PV catch-up queue: PV(c') may lag
                # when a stream starts (pvT2 slot frees only after the
                # previous stream's normalize), then catches up at 2/col.
                pvq = [(16 * s2 + j2, s2, j2)
                       for s2 in range(NS) for j2 in range(NKJ)]
                pvq.reverse()  # pop() from the end
                pv15col = {}
                fin_done = set()
                c = 0
                while pvq:
                    s, j = divmod(c, 16)
                    if s < NS:
                        emit_col(s, j)
                    budget = 2
                    while pvq and budget:
                        ce, s2, j2 = pvq[-1]
                        if ce > c - 2:
                            break
                        if j2 == 0 and s2 >= 1 and parts & 4:
                            if s2 - 1 not in fin_done:
                                # recip needs ~2 columns before bc reads it
                                if s2 - 1 in pv15col and \
                                        c < pv15col[s2 - 1] + 2:
                                    break
                                emit_finalize(s2 - 1)
                                fin_done.add(s2 - 1)
                        pvq.pop()
                        emit_pv(s2, j2)
                        if j2 == NKJ - 1:
                            pv15col[s2] = c
                        budget -= 1
                    if s < NS:
                        fillers(s, j)
                    c += 1
                # drain: last normalize + last chunk's o-proj
                if parts & 4:
                    emit_finalize(NS - 1)
                if parts & 8:
                    for mm in range(4):
                        m16 = (NCH - 1) * 4 + mm
                        emit_oproj_half(m16, 0, tag="sj", bufs=2)
                        emit_oproj_half(m16, 1)

    nc.compile()
    _CACHE[key] = nc
    return nc


def _in_maps(q, k, v, mask, w_q, b_q, w_k, b_k, w_v, b_v, w_o, b_o):
    q = np.asarray(q, dtype=np.float32)
    k = np.asarray(k, dtype=np.float32)
    v = np.asarray(v, dtype=np.float32)
    mask = np.asarray(mask)
    w_q = np.asarray(w_q, dtype=np.float32)
    w_k = np.asarray(w_k, dtype=np.float32)
    w_v = np.asarray(w_v, dtype=np.float32)
    w_o = np.asarray(w_o, dtype=np.float32)
    b_q = np.asarray(b_q, dtype=np.float32)
    b_k = np.asarray(b_k, dtype=np.float32)
    b_v = np.asarray(b_v, dtype=np.float32)

    hf = np.float16
    qT = [np.ascontiguousarray(q[b].T) for b in range(B)]
    kT = [np.ascontiguousarray(k[b].T) for b in range(B)]
    vT = [np.ascontiguousarray(v[b].T) for b in range(B)]
    mkT = [np.ascontiguousarray((~mask[b, 0]).T).astype(hf) for b in range(B)]
    ones2 = np.ones((128, NT, NHL, 2), dtype=hf)

    maps = []
    for c in range(NCORES):
        b, hg = c // HGROUPS, c % HGROUPS
        sl = slice(hg * DLOC, (hg + 1) * DLOC)
        maps.append({
            "qT": qT[b], "kT": kT[b], "vT": vT[b], "maskT": mkT[b],
            "wq": np.ascontiguousarray(w_q[:, sl]),
            "wk": np.ascontiguousarray(w_k[:, sl]),
            "wv": np.ascontiguousarray(w_v[:, sl]),
            "wo": np.ascontiguousarray(w_o[sl, :]),
            "bq": np.ascontiguousarray(b_q[sl].reshape(2, 128).T),
            "bk": np.ascontiguousarray(b_k[sl].reshape(2, 128).T),
            "ones2": ones2,
            "onesc": np.ones((1, DK), dtype=np.float32),
        })
    return maps


def kernel(q, k, v, mask, w_q, b_q, w_k, b_k, w_v, b_v, w_o, b_o):
    from concourse.bass_utils import run_bass_kernel_spmd

    nc = _build()
    maps = _in_maps(q, k, v, mask, w_q, b_q, w_k, b_k, w_v, b_v, w_o, b_o)
    res = run_bass_kernel_spmd(nc, maps, list(range(NCORES)))
    b_o = np.asarray(b_o, dtype=np.float32)
    out = np.zeros((B, S, D), dtype=np.float32)
    for c in range(NCORES):
        out[c // HGROUPS] += res.results[c]["out"]
    out += b_o + (np.asarray(b_v, dtype=np.float32) @
                  np.asarray(w_o, dtype=np.float32))
    return out


# revision 47
# speedup vs baseline: 9.5702x; 1.0856x over previous
"""Trainium2 Bass kernel for nn_MultiHeadAttention (B=2, S=2048, D=1024, H=16).

Sharding (8 cores): batch (2-way) x head-group (4-way).
Core c: batch b=c//4, head-group hg=c%4 (4 heads = 256 of d_model).
Megatron style: Wq/Wk/Wv column-parallel, Wo row-parallel; the 4 partial
outputs per batch are summed on the host (plus b_o).

Per-core device pipeline (all matmuls f32r = TF32-like, 1 cyc/row):
  DMA order tuned for earliest attention start: wk,wq -> k chunks -> q0
  -> mask0 -> wv -> v chunks; later q chunks / masks / wo prefetched
  during attention.

  Attention is a single software-pipelined stream per (query-chunk,
  head-pair): per 128-key tile j: scoresT[kj,qi] via two K=64 matmuls
  packed 2-heads-per-PE-pass (tile_position row strips) into a 2-bank
  PSUM tile (double-buffered), exp on ACT (scale 1/8 folded, fp16 out),
  keep-mask multiply on DVE (fp16 2x, head-broadcast), PV accumulation
  (ones column rides along for row sums) lagging one j behind.

  The ACT engine is the steady-state bottleneck (~1.04us per j), so all
  other work is interleaved into the j-loop as PE filler at fixed j
  positions: normalization of the previous head-pair (reciprocal of the
  PSUM sums row, K=1 outer-product broadcast on PE, TT multiply),
  o-projection of the previous query chunk, and q-projection of the
  next. This keeps PE dense (p-state at full clock) and hides
  everything under the exp stream.
"""
import os

if "JAX_PLATFORMS" in os.environ and "axon" not in os.environ["JAX_PLATFORMS"]:
    del os.environ["JAX_PLATFORMS"]

import numpy as np
import ml_dtypes

B, S, D = 2, 2048, 1024
H, DK = 16, 64
NCORES = 8
HGROUPS = 4               # head-groups (cores per batch)
DLOC = D // HGROUPS       # 256 dims per core
NHL = DLOC // DK          # 4 local heads
NKT = D // 128            # 8 k-tiles over d_model
TCH = 512                 # token chunk
NCH = S // TCH            # 4 chunks
NT = S // 128             # 16 token tiles
NKJ = S // 128            # 16 key tiles
SCALE = 1.0 / 8.0         # 1/sqrt(DK)

# experiment toggles (env-read so bench.py can A/B without edits;
# defaults are the shipping configuration)
MASKMODE = int(os.environ.get("MASKMODE", "0"))  # 0=hh-broadcast 1=per-hh 2=off
RECIPMODE = int(os.environ.get("RECIPMODE", "0"))  # 0=exact 1=approx 2=off
SHIFTPROBE = int(os.environ.get("SHIFTPROBE", "0"))  # 1: timing-only, no
# partition-shifted DVE ops in finalize (hh=1 results land wrong)
IN16 = int(os.environ.get("IN16", "1"))  # 1: q/k/v inputs + weights in bf16
POOLC = int(os.environ.get("POOLC", "0"))  # 1: PSUM->SBUF copies on Pool
# (GPSIMD cannot access PSUM on TRN2 — POOLC=1 fails BIR verification)
MASKPOOL = int(os.environ.get("MASKPOOL", "0"))  # every Nth mask mult on Pool
RAWLATE = int(os.environ.get("RAWLATE", "1"))  # raw aoT copies after recip
ALIGNPV = int(os.environ.get("ALIGNPV", "0"))  # odd heads: ones-cols first,
# PV out at partition offset 62 so normalize copies/mults are aligned

_CACHE = {}


def _build(reps=1, parts=15):
    """Trace + compile the per-core Bass kernel (cached).

    reps>1 wraps the whole body in a tc.For_i hardware loop (timing use).
    parts: bitmask 1=phase1, 2=attention, 4=finalize, 8=oproj (bisection).
    """
    key = ("nc", reps, parts)
    if key in _CACHE:
        return _CACHE[key]
    import concourse.bacc as bacc
    import concourse.bass as bass
    import concourse.mybir as mybir
    from concourse.tile import TileContext

    f32r = mybir.dt.float32r
    f32 = mybir.dt.float32
    f16 = mybir.dt.float16
    bf16 = mybir.dt.bfloat16
    AF = mybir.ActivationFunctionType
    fin = bf16 if IN16 else f32r

    nc = bacc.Bacc("TRN2", target_bir_lowering=False)

    qT_d = nc.dram_tensor("qT", [D, S], fin, kind="ExternalInput")
    kT_d = nc.dram_tensor("kT", [D, S], fin, kind="ExternalInput")
    vT_d = nc.dram_tensor("vT", [D, S], fin, kind="ExternalInput")
    mk_d = nc.dram_tensor("maskT", [S, S], f16, kind="ExternalInput")
    wq_d = nc.dram_tensor("wq", [D, DLOC], fin, kind="ExternalInput")
    wk_d = nc.dram_tensor("wk", [D, DLOC], fin, kind="ExternalInput")
    wv_d = nc.dram_tensor("wv", [D, DLOC], fin, kind="ExternalInput")
    wo_d = nc.dram_tensor("wo", [DLOC, D], f32r, kind="ExternalInput")
    bq_d = nc.dram_tensor("bq", [128, 2], f32r, kind="ExternalInput")
    bk_d = nc.dram_tensor("bk", [128, 2], f32r, kind="ExternalInput")
    ones2_d = nc.dram_tensor("ones2", [128, NT, NHL, 2], f16,
                             kind="ExternalInput")
    onesc_d = nc.dram_tensor("onesc", [1, DK], f32, kind="ExternalInput")
    out_d = nc.dram_tensor("out", [S, D], f32, kind="ExternalOutput")

    qT_r = qT_d.rearrange("(kt p) t -> p kt t", p=128)
    kT_r = kT_d.rearrange("(kt p) t -> p kt t", p=128)
    vT_r = vT_d.rearrange("(kt p) t -> p kt t", p=128)
    mk_r = mk_d.rearrange("(j p) q -> p j q", p=128)

    with TileContext(nc) as tc:
        with (
            tc.tile_pool(name="big", bufs=1) as big,
            tc.tile_pool(name="xin", bufs=2) as xin,
            tc.tile_pool(name="mp", bufs=2) as mp,
            tc.tile_pool(name="ep", bufs=6) as ep,
            tc.tile_pool(name="sp", bufs=2) as sp,
            tc.tile_pool(name="ps", bufs=1, space="PSUM") as ps,
        ):
          import contextlib
          loop_cm = tc.For_i(0, reps, 1) if reps > 1 else contextlib.nullcontext()
          with loop_cm:
            # ---- weights / constants (DMA order = priority order) ----
            wq_sb = big.tile([128, NKT, DLOC], fin)
            wk_sb = big.tile([128, NKT, DLOC], fin)
            wv_sb = big.tile([128, NKT, DLOC], fin)
            wo_sb = big.tile([128, DLOC // 128, D], f32r)
            bq_sb = big.tile([128, 2], f32r)
            bk_sb = big.tile([128, 2], f32r)
            onesc_sb = big.tile([1, DK], f32)
            onesc16_sb = big.tile([1, DK], f16)
            nc.vector.memset(onesc16_sb, 1.0)
            nc.sync.dma_start(out=wk_sb, in_=wk_d.rearrange("(kt p) o -> p kt o", p=128))
            nc.sync.dma_start(out=wq_sb, in_=wq_d.rearrange("(kt p) o -> p kt o", p=128))
            nc.sync.dma_start(out=bk_sb, in_=bk_d[:, :])
            nc.sync.dma_start(out=bq_sb, in_=bq_d[:, :])

            # ---- persistent activations ----
            qhT_sb = big.tile([128, 2, S], f32r)     # [p, m, t]
            khT_sb = big.tile([128, 2, S], f32r)
            vh1_sb = big.tile([128, NT, NHL, DK + 2], f16)
            aoT_sb = big.tile([128, 2, S], f32r)     # normalized attnout^T

            # PSUM budget (8 banks):
            #   sj  tag: [128, 2, TCH] f32 = 2 banks x 2 bufs  (scores,
            #       qk-proj accumulators, normalize broadcast)
            #   pv2 tag: [66, 2, TCH]  f32 = 2 banks x 1 buf   (PV accum)
            #   po  tag: [128, 2, 512] f32 = 2 banks x 1 buf   (v-proj,
            #       o-proj, interleaved q-proj accumulators)

            def emit_x_dma(xname, xr, tch):
                xt = xin.tile([128, NKT, TCH], fin, tag="xt",
                              name=f"xt_{xname}{tch}")
                nc.sync.dma_start(
                    out=xt, in_=xr[:, :, tch * TCH:(tch + 1) * TCH])
                return xt

            def emit_qk_compute(xt, w_sb, b_sb, hT_sb, tch, ms, label,
                                acc_tag="sj", acc_bufs=2):
                acc = ps.tile([128, 2, TCH], f32, tag=acc_tag,
                              name=f"acc_{label}_{tch}_{ms[0]}",
                              bufs=acc_bufs)
                for i, m in enumerate(ms):
                    for kt in range(NKT):
                        nc.tensor.matmul(
                            acc[:, i, :], w_sb[:, kt, m * 128:(m + 1) * 128],
                            xt[:, kt, :],
                            start=(kt == 0), stop=(kt == NKT - 1))
                for i, m in enumerate(ms):
                    nc.vector.tensor_scalar_add(
                        out=hT_sb[:, m, tch * TCH:(tch + 1) * TCH],
                        in0=acc[:, i, :],
                        scalar1=b_sb[:, m:m + 1].bitcast(f32))

            def emit_v_compute(xt, tch):
                for mm in range(TCH // 128):
                    m16 = tch * (TCH // 128) + mm
                    pv = ps.tile([128, 2, 512], f32, tag="po",
                                 name=f"psv_{m16}", bufs=1)
                    for kt in range(NKT):
                        nc.tensor.matmul(
                            pv[:, 0, 0:DLOC],
                            xt[:, kt, mm * 128:(mm + 1) * 128],
                            wv_sb[:, kt, :],
                            start=(kt == 0), stop=(kt == NKT - 1))
                    phd = pv[:, 0, 0:DLOC].rearrange("p (h d) -> p h d",
                                                      h=NHL)
                    if ALIGNPV:
                        nc.vector.tensor_copy(
                            vh1_sb[:, m16, 0::2, 0:DK], phd[:, 0::2, :])
                        nc.vector.tensor_copy(
                            vh1_sb[:, m16, 1::2, 2:DK + 2], phd[:, 1::2, :])
                    else:
                        nc.vector.tensor_copy(vh1_sb[:, m16, :, 0:DK], phd)

            # ---- phase 1: chunk-interleaved so attention can start as
            # soon as (k0, q0, mask0, v0) are resident; all projection
            # accumulators share the serial DMA-paced "po" psum slot so
            # the "sj" slots stay free for the attention scores rhythm.
            mk0_sb = mp.tile([128, NKJ, TCH], f16, tag="mk", name="mk_0")
            if parts & 1:
                kx0 = emit_x_dma("k", kT_r, 0)
                qx0 = emit_x_dma("q", qT_r, 0)
                emit_qk_compute(kx0, wk_sb, bk_sb, khT_sb, 0, [0, 1], "k",
                                acc_tag="po", acc_bufs=1)
                nc.sync.dma_start(out=mk0_sb, in_=mk_r[:, :, 0:TCH])
                nc.sync.dma_start(out=wv_sb,
                                  in_=wv_d.rearrange("(kt p) o -> p kt o", p=128))
                vx0 = emit_x_dma("v", vT_r, 0)
                if ALIGNPV:
                    nc.sync.dma_start(out=vh1_sb[:, :, 0::2, DK:DK + 2],
                                      in_=ones2_d[:, :, 0::2, :])
                    nc.sync.dma_start(out=vh1_sb[:, :, 1::2, 0:2],
                                      in_=ones2_d[:, :, 1::2, :])
                else:
                    nc.sync.dma_start(out=vh1_sb[:, :, :, DK:DK + 2],
                                      in_=ones2_d[:, :, :, :])
                nc.sync.dma_start(out=onesc_sb, in_=onesc_d[:, :])
                emit_qk_compute(qx0, wq_sb, bq_sb, qhT_sb, 0, [0, 1], "q",
                                acc_tag="po", acc_bufs=1)
                emit_v_compute(vx0, 0)
                for tch in range(1, NCH):
                    kxt = emit_x_dma("k", kT_r, tch)
                    vxt = emit_x_dma("v", vT_r, tch)
                    emit_qk_compute(kxt, wk_sb, bk_sb, khT_sb, tch, [0, 1],
                                    "k", acc_tag="po", acc_bufs=1)
                    emit_v_compute(vxt, tch)
            else:
                nc.sync.dma_start(out=mk0_sb, in_=mk_r[:, :, 0:TCH])
                nc.sync.dma_start(out=wv_sb,
                                  in_=wv_d.rearrange("(kt p) o -> p kt o", p=128))
                nc.sync.dma_start(out=vh1_sb[:, :, :, DK:DK + 2],
                                  in_=ones2_d[:, :, :, :])
                nc.sync.dma_start(out=onesc_sb, in_=onesc_d[:, :])

            nc.sync.dma_start(out=wo_sb, in_=wo_d.rearrange("(kk p) o -> p kk o", p=128))

            # ---- attention: one flat pipeline over (stream, j) columns
            # so the exp stream never drains at head-pair boundaries.
            # streams s = (tcq, hp); per column c = 16*s + j:
            #   scores/exp/mask(c), then lag-2 PV(c-2), then PE fillers
            #   (normalize of s-1, o-proj of tcq-1, q-proj of tcq+1).
            streams = [(tcq, hp) for tcq in range(NCH) for hp in range(2)]
            NS = len(streams)
            mk_tiles = {0: mk0_sb}
            state = {"qxt": None}
            pvT2s = {}
            e_tiles = {}

            def emit_finalize(s):
                # off-critical-path normalize: K=1 outer-product broadcast
                # of the reciprocal sums row on PE, then one in-place TT
                # multiply per head on the raw copy already in aoT_sb.
                # (pvT2 was released back at PV(s,15) by the raw copy.)
                tcq, hp = streams[s]
                qsl = slice(tcq * TCH, (tcq + 1) * TCH)
                rec = state.pop(("rec", s))
                ones_lhs = onesc16_sb if RECIPMODE == 0 else onesc_sb
                if ALIGNPV:
                    bc = ps.tile([128, TCH], f32, tag="po",
                                 name=f"bc_{tcq}_{hp}", bufs=1)
                    for hh in range(2):
                        nc.tensor.matmul(bc[64 * hh:64 * hh + DK, :],
                                         ones_lhs, rec[0:1, hh, :],
                                         start=True, stop=True)
                    for hh in range(2):
                        asl = aoT_sb[64 * hh:64 * hh + DK, hp, qsl]
                        nc.vector.tensor_mul(asl, asl,
                                             bc[64 * hh:64 * hh + DK, :])
                else:
                    bc = ps.tile([DK, 2, TCH], f32, tag="po",
                                 name=f"bc_{tcq}_{hp}", bufs=1)
                    for hh in range(2):
                        nc.tensor.matmul(bc[:, hh, :], ones_lhs,
                                         rec[0:1, hh, :],
                                         start=True, stop=True)
                    for hh in range(2):
                        sh = 0 if SHIFTPROBE else 64 * hh
                        asl = aoT_sb[sh:sh + DK, hp, qsl]
                        nc.vector.tensor_mul(asl, asl, bc[:, hh, :])

            def emit_oproj_half(m16, half, tag="po", bufs=1):
                # split across two filler columns to stay inside the
                # per-column PE slack
                if half == 0:
                    po = ps.tile([128, 2, 512], f32, tag=tag,
                                 name=f"po_{m16}", bufs=bufs)
                    state[("po", m16)] = po
                else:
                    po = state.pop(("po", m16))
                for kk in range(2):
                    nc.tensor.matmul(
                        po[:, half, :],
                        aoT_sb[:, kk, m16 * 128:(m16 + 1) * 128],
                        wo_sb[:, kk, half * 512:(half + 1) * 512],
                        start=(kk == 0), stop=(kk == 1))
                if half == 1:
                    o_sb = sp.tile([128, D], f32, tag="o", name=f"o_{m16}",
                                   bufs=2)
                    eng = nc.gpsimd if POOLC else nc.vector
                    eng.tensor_copy(
                        o_sb.rearrange("p (n q) -> p n q", n=2), po)
                    nc.sync.dma_start(
                        out=out_d[m16 * 128:(m16 + 1) * 128, :], in_=o_sb)

            def emit_col(s, j):
                tcq, hp = streams[s]
                qsl = slice(tcq * TCH, (tcq + 1) * TCH)
                s_j = ps.tile([128, 2, TCH], f32, tag="sj",
                              name=f"s_{tcq}_{hp}_{j}", bufs=2)
                for hh in range(2):
                    nc.tensor.matmul(
                        s_j[:, hh, :],
                        khT_sb[64 * hh:64 * (hh + 1), hp,
                               j * 128:(j + 1) * 128],
                        qhT_sb[64 * hh:64 * (hh + 1), hp, qsl],
                        start=True, stop=True,
                        tile_position=(64 * hh, 0))
                e_sb = ep.tile([128, 2, TCH], f16, tag="e",
                               name=f"e_{tcq}_{hp}_{j}", bufs=8)
                e_tiles[16 * s + j] = e_sb
                nc.scalar.activation(
                    out=e_sb, in_=s_j, func=AF.Exp, scale=SCALE)
                mk_sb = mk_tiles[tcq]
                msl = mk_sb[:, j, :]
                if MASKMODE == 0:
                    mbc = bass.AP(
                        tensor=msl.tensor, offset=msl.offset,
                        ap=[msl.ap[0], [0, 2], msl.ap[1]])
                    eng = (nc.gpsimd if MASKPOOL and j % MASKPOOL == 0
                           else nc.vector)
                    eng.tensor_mul(e_sb, e_sb, mbc)
                elif MASKMODE == 1:
                    for hh in range(2):
                        nc.vector.tensor_mul(
                            e_sb[:, hh, :], e_sb[:, hh, :], msl)

            def emit_pv(s, j):
                tcq, hp = streams[s]
                if j == 0:
                    pvT2s[s] = ps.tile([128, 2, TCH], f32, tag="pv2",
                                       name=f"pvT_{tcq}_{hp}", bufs=1)
                pvT2 = pvT2s[s]
                e_c = e_tiles.pop(16 * s + j)
                for hh in range(2):
                    if ALIGNPV:
                        # odd heads have [one,one,v..] stationary layout;
                        # output lands at partitions 62..127 so the data
                        # rows (64..127) line up with aoT's partitions
                        tgt = (pvT2[0:DK + 2, 0, :] if hh == 0
                               else pvT2[DK - 2:128, 1, :])
                    else:
                        tgt = pvT2[0:DK + 2, hh, :]
                    nc.tensor.matmul(
                        tgt,
                        vh1_sb[:, j, hp * 2 + hh, :],
                        e_c[:, hh, :],
                        start=(j == 0), stop=(j == NKJ - 1))
                if j == NKJ - 1 and parts & 4:
                    # free the single-buffered pvT2 slot ASAP: the raw
                    # copies into aoT_sb + the sums-row read are the only
                    # pvT2 readers, and DVE runs in emission order — so
                    # the copies go FIRST and the reciprocal (off the
                    # release path) last. Normalize multiply happens
                    # later in place (emit_finalize).
                    qsl = slice(tcq * TCH, (tcq + 1) * TCH)

                    def raw_copies():
                        for hh in range(2):
                            sh = 0 if SHIFTPROBE else 64 * hh
                            if ALIGNPV:
                                nc.vector.tensor_copy(
                                    aoT_sb[sh:sh + DK, hp, qsl],
                                    pvT2[64 * hh:64 * hh + DK, hh, :])
                            else:
                                nc.vector.tensor_copy(
                                    aoT_sb[sh:sh + DK, hp, qsl],
                                    pvT2[0:DK, hh, :])

                    if not RAWLATE:
                        raw_copies()
                    if RECIPMODE == 0:
                        # fp32 reciprocal costs ~6us/stream on HW (DVE
                        # becomes the pacing engine); fp16 halves the
                        # element bytes and keeps ~11-bit accuracy, ample
                        # for softmax normalization.
                        s16 = sp.tile([1, 2, TCH], f16, tag="s16",
                                      name=f"s16_{tcq}_{hp}", bufs=2)
                        if ALIGNPV:
                            nc.vector.tensor_copy(
                                s16[:, 0, :], pvT2[DK:DK + 1, 0, :])
                            nc.vector.tensor_copy(
                                s16[:, 1, :], pvT2[DK - 2:DK - 1, 1, :])
                        else:
                            nc.vector.tensor_copy(s16, pvT2[DK:DK + 1, :, :])
                        rec = sp.tile([1, 2, TCH], f16, tag="rec",
                                      name=f"rec_{tcq}_{hp}")
                        with nc.allow_low_precision(reason="softmax denom"):
                            nc.vector.reciprocal(rec, s16)
                    else:
                        assert not ALIGNPV, "RECIPMODE!=0 needs ALIGNPV=0"
                        rec = sp.tile([1, 2, TCH], f32, tag="rec",
                                      name=f"rec_{tcq}_{hp}")
                        with nc.allow_low_precision(reason="recip"):
                            nc.vector.reciprocal(rec, pvT2[DK:DK + 1, :, :])
                    state[("rec", s)] = rec
                    if RAWLATE:
                        raw_copies()
                    pvT2s.pop(s)

            def fillers(s, j):
                tcq, hp = streams[s]
                if j == 4 and s >= 1 and parts & 4:
                    emit_finalize(s - 1)
                if hp == 0 and j == 14 and tcq + 1 < NCH:
                    mk = mp.tile([128, NKJ, TCH], f16, tag="mk",
                                 name=f"mk_{tcq + 1}")
                    nc.sync.dma_start(
                        out=mk,
                        in_=mk_r[:, :, (tcq + 1) * TCH:(tcq + 2) * TCH])
                    mk_tiles[tcq + 1] = mk
                if parts & 8 and j in (6, 7, 10, 11) and tcq > 0:
                    emit_oproj_half((tcq - 1) * 4 + hp * 2 + (j >= 10),
                                    half=int(j in (7, 11)))
                if parts & 1 and j == 9 and hp == 0 and tcq + 1 < NCH:
                    state["qxt"] = emit_x_dma("q", qT_r, tcq + 1)
                if parts & 1 and j >= 12 and tcq + 1 < NCH:
                    # 2 of the 8 K-tiles per column, same accumulation group
                    kts = range(2 * (j - 12), 2 * (j - 12) + 2)
                    emit_qk_part(state["qxt"], tcq + 1, hp, kts, j == 12)

            def emit_qk_part(xt, tch, m, kts, first):
                if first:
                    state[("qacc", m)] = ps.tile(
                        [128, 2, TCH], f32, tag="po",
                        name=f"qacc_{tch}_{m}", bufs=1)
                acc = state[("qacc", m)]
                for kt in kts:
                    nc.tensor.matmul(
                        acc[:, 0, :], wq_sb[:, kt, m * 128:(m + 1) * 128],
                        xt[:, kt, :],
                        start=(kt == 0), stop=(kt == NKT - 1))
                if kts[-1] == NKT - 1:
                    state.pop(("qacc", m))
                    nc.vector.tensor_scalar_add(
                        out=qhT_sb[:, m, tch * TCH:(tch + 1) * TCH],
                        in0=acc[:, 0, :],
                        scalar1=bq_sb[:, m:m + 1].bitcast(f32))

            if parts & 2:
                # flat pipeline with a PV catch-up queue: PV(c') may lag
                # when a stream starts (pvT2 slot frees only after the
                # previous stream's normalize), then catches up at 2/col.
                pvq = [(16 * s2 + j2, s2, j2)
                       for s2 in range(NS) for j2 in range(NKJ)]
                pvq.reverse()  # pop() from the end
                c = 0
                while pvq:
                    s, j = divmod(c, 16)
                    if s < NS:
                        emit_col(s, j)
                    budget = 2
                    while pvq and budget:
                        ce, s2, j2 = pvq[-1]
                        if ce > c - 2:
                            break
                        pvq.pop()
                        emit_pv(s2, j2)
                        budget -= 1
                    if s < NS:
                        fillers(s, j)
                    c += 1
                # drain: last normalize + last chunk's o-proj
                if parts & 4:
                    emit_finalize(NS - 1)
                if parts & 8:
                    for mm in range(4):
                        m16 = (NCH - 1) * 4 + mm
                        emit_oproj_half(m16, 0, tag="sj", bufs=2)
                        emit_oproj_half(m16, 1)

    nc.compile()
    _CACHE[key] = nc
    return nc


def _in_maps(q, k, v, mask, w_q, b_q, w_k, b_k, w_v, b_v, w_o, b_o):
    q = np.asarray(q, dtype=np.float32)
    k = np.asarray(k, dtype=np.float32)
    v = np.asarray(v, dtype=np.float32)
    mask = np.asarray(mask)
    w_q = np.asarray(w_q, dtype=np.float32)
    w_k = np.asarray(w_k, dtype=np.float32)
    w_v = np.asarray(w_v, dtype=np.float32)
    w_o = np.asarray(w_o, dtype=np.float32)
    b_q = np.asarray(b_q, dtype=np.float32)
    b_k = np.asarray(b_k, dtype=np.float32)
    b_v = np.asarray(b_v, dtype=np.float32)

    hf = np.float16
    bfin = ml_dtypes.bfloat16 if IN16 else np.float32
    qT = [np.ascontiguousarray(q[b].T).astype(bfin) for b in range(B)]
    kT = [np.ascontiguousarray(k[b].T).astype(bfin) for b in range(B)]
    vT = [np.ascontiguousarray(v[b].T).astype(bfin) for b in range(B)]
    mkT = [np.ascontiguousarray((~mask[b, 0]).T).astype(hf) for b in range(B)]
    ones2 = np.ones((128, NT, NHL, 2), dtype=hf)

    maps = []
    for c in range(NCORES):
        b, hg = c // HGROUPS, c % HGROUPS
        sl = slice(hg * DLOC, (hg + 1) * DLOC)
        maps.append({
            "qT": qT[b], "kT": kT[b], "vT": vT[b], "maskT": mkT[b],
            "wq": np.ascontiguousarray(w_q[:, sl]).astype(bfin),
            "wk": np.ascontiguousarray(w_k[:, sl]).astype(bfin),
            "wv": np.ascontiguousarray(w_v[:, sl]).astype(bfin),
            "wo": np.ascontiguousarray(w_o[sl, :]),
            "bq": np.ascontiguousarray(b_q[sl].reshape(2, 128).T),
            "bk": np.ascontiguousarray(b_k[sl].reshape(2, 128).T),
            "ones2": ones2,
            "onesc": np.ones((1, DK), dtype=np.float32),
        })
    return maps


def kernel(q, k, v, mask, w_q, b_q, w_k, b_k, w_v, b_v, w_o, b_o):
    from concourse.bass_utils import run_bass_kernel_spmd

    nc = _build()
    maps = _in_maps(q, k, v, mask, w_q, b_q, w_k, b_k, w_v, b_v, w_o, b_o)
    res = run_bass_kernel_spmd(nc, maps, list(range(NCORES)))
    b_o = np.asarray(b_o, dtype=np.float32)
    out = np.zeros((B, S, D), dtype=np.float32)
    for c in range(NCORES):
        out[c // HGROUPS] += res.results[c]["out"]
    out += b_o + (np.asarray(b_v, dtype=np.float32) @
                  np.asarray(w_o, dtype=np.float32))
    return out
